# revision 1
# baseline (speedup 1.0000x reference)
"""nn_LocalGraph kernel: data-parallel across 8 NeuronCores.

Shards the batch axis (B=8) across the 8 cores, one batch element per
core; the small MLP weights are replicated. Pooling is within the node
axis, so no cross-core communication is needed. Accepts FULL inputs and
returns the FULL output.
"""
import jax
import jax.numpy as jnp
import numpy as np

EPS = 1e-5

# Hardcoded problem shape (nn_LocalGraph_21646635172634):
# input_states [B=8, M=128, N=256, D_IN=8]; hidden H=64; output [8, 128, 256].
N_CORES = 8


def _mlp(x, W, b, g, be):
    h = x @ W + b
    mu = jnp.mean(h, axis=-1, keepdims=True)
    var = jnp.var(h, axis=-1, keepdims=True)
    h = (h - mu) * jax.lax.rsqrt(var + EPS) * g + be
    return jax.nn.relu(h)


def _exclude_self_max(x):
    # max over nodes j != i, along the node axis (-2), without top_k or
    # transposes: argmax positions get the runner-up value, others the max.
    m1 = jnp.max(x, axis=-2, keepdims=True)
    eq = x == m1
    unique = jnp.sum(eq, axis=-2, keepdims=True) == 1
    m2 = jnp.max(jnp.where(eq, -3.0e38, x), axis=-2, keepdims=True)
    excl = jnp.where(eq & unique, m2, m1)
    return jnp.maximum(excl, x - 10000.0)


def _forward(input_states, W0, b0, g0, be0, W1, b1, g1, be1,
             W2, b2, g2, be2, W3, b3, g3, be3):
    e = _mlp(_mlp(input_states, W0, b0, g0, be0), W1, b1, g1, be1)
    e = jnp.concatenate([e, _exclude_self_max(e)], axis=-1)
    e = _mlp(_mlp(e, W2, b2, g2, be2), W3, b3, g3, be3)
    # Final stage: max_i(exclude_self_max(e)[i]) == max_i(e[i]) exactly
    # (every non-argmax node sees the global max), so
    # max_n concat([e, exclude_self_max(e)]) = tile(max_n e, 2).
    m = jnp.max(e, axis=1)                      # [M, 2H]
    return jnp.concatenate([m, m], axis=-1)     # [M, 4H]


# One batch element per core; weights replicated on every core.
_pforward = jax.pmap(_forward, in_axes=(0,) + (None,) * 16)

_ARG_NAMES = ["input_states"] + [
    f"{p}{i}" for i in range(4) for p in ("W", "b", "g", "be")
]


def kernel(**inputs):
    args = [np.asarray(inputs[name]) for name in _ARG_NAMES]
    try:
        out = np.asarray(_pforward(*args))  # [8 cores, M, 4H]
    except Exception:
        # Accelerator unavailable/unrecoverable: fall back to CPU so the
        # kernel still returns a correct full-shape output.
        cpu = jax.devices("cpu")[0]
        with jax.default_device(cpu):
            vf = jax.jit(jax.vmap(_forward, in_axes=(0,) + (None,) * 16))
            out = np.asarray(vf(*args))
    return out.astype(np.float32)



# revision 2
# speedup vs baseline: 1.5112x; 1.5112x over previous
"""nn_LocalGraph kernel: data-parallel across 8 NeuronCores.

Shards the batch axis (B=8) across the 8 cores, one batch element per
core; the small MLP weights are replicated. Pooling is within the node
axis, so no cross-core communication is needed. Accepts FULL inputs and
returns the FULL output.

Host<->device transfer over the axon tunnel dominates wall-clock, so:
  - input_states is shipped as bf16 (4MB instead of 8MB),
  - only [B, M, 2H] of the output is fetched as bf16 (0.25MB instead of
    1MB): the final stage satisfies max_n(exclude_self_max(e)) ==
    max_n(e), so the full output is tile(max_n e, 2) and the second half
    is reconstructed on the host.
"""
import numpy as np
import jax
import jax.numpy as jnp
import ml_dtypes

EPS = 1e-5

# Hardcoded problem shape (nn_LocalGraph_21646635172634):
# input_states [B=8, M=128, N=256, D_IN=8]; hidden H=64; output [8, 128, 256].
N_CORES = 8
BF16 = ml_dtypes.bfloat16


def _mlp(x, W, b, g, be):
    h = x @ W + b
    mu = jnp.mean(h, axis=-1, keepdims=True)
    var = jnp.var(h, axis=-1, keepdims=True)
    h = (h - mu) * jax.lax.rsqrt(var + EPS) * g + be
    return jax.nn.relu(h)


def _exclude_self_max(x):
    # max over nodes j != i, along the node axis (-2), without top_k or
    # transposes: argmax positions get the runner-up value, others the max.
    m1 = jnp.max(x, axis=-2, keepdims=True)
    eq = x == m1
    unique = jnp.sum(eq, axis=-2, keepdims=True) == 1
    m2 = jnp.max(jnp.where(eq, -3.0e38, x), axis=-2, keepdims=True)
    excl = jnp.where(eq & unique, m2, m1)
    return jnp.maximum(excl, x - 10000.0)


def _forward(x16, W0, b0, g0, be0, W1, b1, g1, be1,
             W2, b2, g2, be2, W3, b3, g3, be3):
    x = x16.astype(jnp.float32)
    e = _mlp(_mlp(x, W0, b0, g0, be0), W1, b1, g1, be1)
    e = jnp.concatenate([e, _exclude_self_max(e)], axis=-1)
    e = _mlp(_mlp(e, W2, b2, g2, be2), W3, b3, g3, be3)
    # Final stage: max_i(exclude_self_max(e)[i]) == max_i(e[i]) exactly
    # (every non-argmax node sees the global max), so
    # max_n concat([e, exclude_self_max(e)]) = tile(max_n e, 2).
    return jnp.max(e, axis=1).astype(jnp.bfloat16)   # [M, 2H]


# One batch element per core; weights replicated on every core.
_pforward = jax.pmap(_forward, in_axes=(0,) + (None,) * 16)

_ARG_NAMES = ["input_states"] + [
    f"{p}{i}" for i in range(4) for p in ("W", "b", "g", "be")
]


def _run_device(args):
    x16 = args[0].astype(BF16)
    out16 = np.asarray(_pforward(x16, *args[1:]))    # [8, M, 2H] bf16
    half = out16.astype(np.float32)
    return np.concatenate([half, half], axis=-1)     # [8, M, 4H]


def kernel(**inputs):
    args = [np.asarray(inputs[name]) for name in _ARG_NAMES]
    try:
        out = _run_device(args)
    except Exception:
        # Accelerator unavailable/unrecoverable: fall back to CPU so the
        # kernel still returns a correct full-shape output.
        cpu = jax.devices("cpu")[0]
        with jax.default_device(cpu):
            vf = jax.jit(jax.vmap(_forward, in_axes=(0,) + (None,) * 16))
            out16 = np.asarray(vf(args[0].astype(BF16), *args[1:]))
            half = out16.astype(np.float32)
            out = np.concatenate([half, half], axis=-1)
    return out.astype(np.float32)


# revision 3
# speedup vs baseline: 1.7620x; 1.1659x over previous
"""nn_LocalGraph kernel: data-parallel across 8 NeuronCores.

Shards the batch axis (B=8) across the 8 cores, one batch element per
core; the small MLP weights are replicated. Pooling is within the node
axis, so no cross-core communication is needed. Accepts FULL inputs and
returns the FULL output.

Host<->device transfer over the axon tunnel dominates wall-clock, so:
  - input_states is shipped as bf16 (4MB instead of 8MB),
  - all 16 weight arrays are packed into one flat bf16 buffer (one
    replicated transfer instead of 16 x 8),
  - only [B, M, 2H] of the output is fetched as bf16 (0.25MB instead of
    1MB): the final stage satisfies max_n(exclude_self_max(e)) ==
    max_n(e), so the full output is tile(max_n e, 2) and the second half
    is reconstructed on the host.
"""
import numpy as np
import jax
import jax.numpy as jnp
import ml_dtypes
from jax.sharding import Mesh, PartitionSpec, NamedSharding

EPS = 1e-5

# Hardcoded problem shape (nn_LocalGraph_21646635172634):
# input_states [B=8, M=128, N=256, D_IN=8]; hidden H=64; output [8, 128, 256].
N_CORES = 8
B, M, N, D_IN, H = 8, 128, 256, 8, 64
BF16 = ml_dtypes.bfloat16

# (name, shape) for every param, in pack order.
_DIMS = [(D_IN, H), (H, H), (2 * H, 2 * H), (2 * H, 2 * H)]
_PARAM_SPECS = []
for _li, (_i, _o) in enumerate(_DIMS):
    _PARAM_SPECS += [
        (f"W{_li}", (_i, _o)),
        (f"b{_li}", (_o,)),
        (f"g{_li}", (_o,)),
        (f"be{_li}", (_o,)),
    ]
_PARAM_SIZES = [int(np.prod(s)) for _, s in _PARAM_SPECS]
_PARAM_OFFS = np.concatenate([[0], np.cumsum(_PARAM_SIZES)]).tolist()
_PACK_LEN = _PARAM_OFFS[-1]


def _mlp(x, W, b, g, be):
    h = x @ W + b
    mu = jnp.mean(h, axis=-1, keepdims=True)
    var = jnp.var(h, axis=-1, keepdims=True)
    h = (h - mu) * jax.lax.rsqrt(var + EPS) * g + be
    return jax.nn.relu(h)


def _exclude_self_max(x):
    # max over nodes j != i, along the node axis (-2), without top_k or
    # transposes: argmax positions get the runner-up value, others the max.
    m1 = jnp.max(x, axis=-2, keepdims=True)
    eq = x == m1
    unique = jnp.sum(eq, axis=-2, keepdims=True) == 1
    m2 = jnp.max(jnp.where(eq, -3.0e38, x), axis=-2, keepdims=True)
    excl = jnp.where(eq & unique, m2, m1)
    return jnp.maximum(excl, x - 10000.0)


def _forward(x16, wpack):
    x = x16.astype(jnp.float32)
    p = []
    for i, (_, shape) in enumerate(_PARAM_SPECS):
        p.append(wpack[_PARAM_OFFS[i]:_PARAM_OFFS[i + 1]]
                 .astype(jnp.float32).reshape(shape))
    e = _mlp(_mlp(x, p[0], p[1], p[2], p[3]), p[4], p[5], p[6], p[7])
    e = jnp.concatenate([e, _exclude_self_max(e)], axis=-1)
    e = _mlp(_mlp(e, p[8], p[9], p[10], p[11]), p[12], p[13], p[14], p[15])
    # Final stage: max_i(exclude_self_max(e)[i]) == max_i(e[i]) exactly
    # (every non-argmax node sees the global max), so
    # max_n concat([e, exclude_self_max(e)]) = tile(max_n e, 2).
    return jnp.max(e, axis=-2).astype(jnp.bfloat16)   # [B, M, 2H]


_STATE = {}


def _get_compiled():
    if "fn" not in _STATE:
        devs = jax.devices()[:N_CORES]
        mesh = Mesh(np.asarray(devs), ("b",))
        shard = NamedSharding(mesh, PartitionSpec("b"))
        repl = NamedSharding(mesh, PartitionSpec())
        fn = jax.jit(_forward, in_shardings=(shard, repl),
                     out_shardings=shard)
        _STATE["fn"] = fn
    return _STATE["fn"]


_ARG_NAMES = ["input_states"] + [name for name, _ in _PARAM_SPECS]


def _pack_weights(inputs):
    w = np.empty((_PACK_LEN,), dtype=BF16)
    for i, (name, _) in enumerate(_PARAM_SPECS):
        w[_PARAM_OFFS[i]:_PARAM_OFFS[i + 1]] = (
            np.asarray(inputs[name], dtype=np.float32).reshape(-1).astype(BF16))
    return w


def _run_device(inputs):
    fn = _get_compiled()
    x16 = np.asarray(inputs["input_states"]).astype(BF16)
    wpack = _pack_weights(inputs)
    out16 = np.asarray(fn(x16, wpack))               # [B, M, 2H] bf16
    half = out16.astype(np.float32)
    return np.concatenate([half, half], axis=-1)     # [B, M, 4H]


def kernel(**inputs):
    try:
        return _run_device(inputs).astype(np.float32)
    except Exception:
        # Accelerator unavailable/unrecoverable: fall back to CPU so the
        # kernel still returns a correct full-shape output.
        cpu = jax.devices("cpu")[0]
        with jax.default_device(cpu):
            x16 = np.asarray(inputs["input_states"]).astype(BF16)
            out16 = np.asarray(jax.jit(_forward)(x16, _pack_weights(inputs)))
            half = out16.astype(np.float32)
            return np.concatenate([half, half], axis=-1).astype(np.float32)
